# revision 8
# baseline (speedup 1.0000x reference)
import math
from contextlib import ExitStack

import numpy as np

from concourse import bass, tile, mybir, bacc, masks
from concourse import bass_utils

H, N, V, QL = 8, 512, 64, 64
VH = V * H
B, S = 32, 512
NCORES = 8
NS = N // NCORES            # 64 neurons per core
NCHUNK = VH // 128          # 4 contraction chunks
PRE = 12                    # weight prefetch depth (tiles)
EPS = 1e-12

_PROGRAMS = {}
LAST_RESULTS = None


def _build(self_conn: bool):
    f16, f32 = mybir.dt.float16, mybir.dt.float32
    Exp = mybir.ActivationFunctionType.Exp
    Sqrt = mybir.ActivationFunctionType.Sqrt

    nc = bacc.Bacc("TRN2", target_bir_lowering=False, debug=False, num_devices=NCORES)
    hkT = nc.dram_tensor("hkT", [QL, B, S], f16, kind="ExternalInput").ap()
    hv = nc.dram_tensor("hv", [B, S, VH], f16, kind="ExternalInput").ap()
    qbT = nc.dram_tensor("qbT", [QL, H * NS], f16, kind="ExternalInput").ap()
    sgT = None
    if self_conn:
        sgT = nc.dram_tensor("sgT", [S, H * NS], f16, kind="ExternalInput").ap()
    wv = nc.dram_tensor("wv", [NS, 128, NCHUNK * VH], f16, kind="ExternalInput").ap()
    wk = nc.dram_tensor("wk", [NS, 128, NCHUNK * QL], f16, kind="ExternalInput").ap()
    ovo = nc.dram_tensor("ovo", [NS, B, VH], f32, kind="ExternalOutput").ap()
    oko = nc.dram_tensor("oko", [NS, B, QL], f32, kind="ExternalOutput").ap()

    hvr = hv.rearrange("b (c p) v -> p (b c) v", p=128)

    with tile.TileContext(nc) as tc:
        with ExitStack() as ctx:
            singles = ctx.enter_context(tc.tile_pool(name="singles", bufs=1))
            hkp = ctx.enter_context(tc.tile_pool(name="hkp", bufs=2))
            apool = ctx.enter_context(tc.tile_pool(name="apool", bufs=2))
            epool = ctx.enter_context(tc.tile_pool(name="epool", bufs=2))
            txp = ctx.enter_context(tc.tile_pool(name="txp", bufs=2))
            xbp = ctx.enter_context(tc.tile_pool(name="xbp", bufs=2))
            smp = ctx.enter_context(tc.tile_pool(name="smp", bufs=2))
            wvp = ctx.enter_context(tc.tile_pool(name="wvp", bufs=PRE))
            wkp = ctx.enter_context(tc.tile_pool(name="wkp", bufs=PRE))
            opool = ctx.enter_context(tc.tile_pool(name="opool", bufs=4))

            qb_sb = singles.tile([QL, H * NS], f16)
            nc.sync.dma_start(out=qb_sb[:], in_=qbT[:])
            if self_conn:
                sg_sb = singles.tile([128, NCHUNK, H * NS], f16)
                nc.sync.dma_start(out=sg_sb[:], in_=sgT.rearrange("(c p) n -> p c n", p=128))
            ident = singles.tile([64, 64], f16)
            masks.make_identity(nc, ident[:])
            eps_sb = singles.tile([64, 1], f32)
            nc.vector.memset(eps_sb[:], EPS)
            xt = [singles.tile([128, B * NS], f16, name=f"xt{c}") for c in range(NCHUNK)]

            wtiles = {}

            def load_w(n):
                twv = wvp.tile([128, NCHUNK * VH], f16)
                nc.gpsimd.dma_start(out=twv[:], in_=wv[n])
                twk = wkp.tile([128, NCHUNK * QL], f16)
                nc.gpsimd.dma_start(out=twk[:], in_=wk[n])
                wtiles[n] = (twv, twk)

            for n in range(min(PRE, NS)):
                load_w(n)

            # ---- phase 1: attention + custom softmax + layernorm, per batch ----
            with tc.tile_pool(name="scps", bufs=2, space="PSUM") as scps, \
                 tc.tile_pool(name="atps", bufs=1, space="PSUM") as atps, \
                 tc.tile_pool(name="tpps", bufs=2, space="PSUM") as tpps:
                for b in range(B):
                    hk_sb = hkp.tile([QL, S], f16)
                    nc.sync.dma_start(out=hk_sb[:], in_=hkT[:, b, :])
                    a_sb = apool.tile([128, NCHUNK, 520], f16)
                    nc.sync.dma_start(out=a_sb[:, :, 0:VH], in_=hvr[:, b * 4:b * 4 + 4, :])
                    nc.vector.memset(a_sb[:, :, VH:520], 1.0)

                    e_sb = epool.tile([128, NCHUNK, H * NS], f16)
                    for j in range(2):
                        sc = scps.tile([128, 2, H * NS], f32)
                        for jj in range(2):
                            c = 2 * j + jj
                            nc.tensor.matmul(sc[:, jj, :], hk_sb[:, c * 128:(c + 1) * 128],
                                             qb_sb[:], start=True, stop=True)
                        if self_conn:
                            tex = txp.tile([128, 2, H * NS], f16)
                            nc.scalar.activation(out=tex[:], in_=sc[:], func=Exp)
                            nc.vector.tensor_mul(e_sb[:, 2 * j:2 * j + 2, :], tex[:],
                                                 sg_sb[:, 2 * j:2 * j + 2, :])
                        else:
                            nc.scalar.activation(out=e_sb[:, 2 * j:2 * j + 2, :], in_=sc[:],
                                                 func=Exp)

                    at_a = atps.tile([64, 4, V + 1], f32)
                    at_b = atps.tile([64, 4, V + 1], f32)
                    av = a_sb[:].rearrange("p c (v h) -> p c h v", h=H)
                    for h in range(H):
                        at = at_a if h < 4 else at_b
                        for c in range(NCHUNK):
                            nc.tensor.matmul(at[:, h % 4, :],
                                             e_sb[:, c, h * NS:(h + 1) * NS],
                                             av[:, c, h, :],
                                             start=(c == 0), stop=(c == NCHUNK - 1))

                    recs = smp.tile([64, H], f32)
                    nc.vector.reciprocal(recs[:, 0:4],
                                         at_a[:, :, V:V + 1].rearrange("p c o -> p (c o)"))
                    nc.vector.reciprocal(recs[:, 4:8],
                                         at_b[:, :, V:V + 1].rearrange("p c o -> p (c o)"))
                    x_b = xbp.tile([64, VH], f32)
                    xv = x_b[:].rearrange("p (v h) -> p h v", h=H)
                    r0 = recs[:]
                    ra = bass.AP(tensor=r0.tensor, offset=r0.offset, ap=[[H, 64], [1, 4], [0, V]])
                    rb = bass.AP(tensor=r0.tensor, offset=r0.offset + 4,
                                 ap=[[H, 64], [1, 4], [0, V]])
                    nc.vector.tensor_mul(xv[:, 0:4, :], at_a[:, :, 0:V], ra)
                    nc.vector.tensor_mul(xv[:, 4:8, :], at_b[:, :, 0:V], rb)

                    st6 = smp.tile([64, 6], f32)
                    mv = smp.tile([64, 2], f32)
                    nc.vector.bn_stats(st6[:], x_b[:])
                    nc.vector.bn_aggr(mv[:], st6[:])
                    rstd = smp.tile([64, 1], f32)
                    nc.scalar.activation(out=rstd[:], in_=mv[:, 1:2], func=Sqrt, bias=eps_sb[:])
                    nc.vector.reciprocal(rstd[:], rstd[:])
                    x16 = xbp.tile([64, VH], f16)
                    nc.vector.tensor_scalar(out=x16[:], in0=x_b[:], scalar1=mv[:, 0:1],
                                            scalar2=rstd[:], op0=mybir.AluOpType.subtract,
                                            op1=mybir.AluOpType.mult)
                    for c in range(NCHUNK):
                        pst = tpps.tile([128, 64], f16)
                        nc.tensor.transpose(pst[:], x16[:, c * 128:(c + 1) * 128], ident[:])
                        nc.vector.tensor_copy(out=xt[c][:, b * NS:(b + 1) * NS], in_=pst[:])

            # ---- phase 2: per-neuron linears ----
            with tc.tile_pool(name="pvps", bufs=2, space="PSUM") as pvps, \
                 tc.tile_pool(name="pkps", bufs=2, space="PSUM") as pkps:
                for n in range(NS):
                    twv, twk = wtiles.pop(n)
                    pv = pvps.tile([B, VH], f32)
                    pk = pkps.tile([B, QL], f32)
                    for c in range(NCHUNK):
                        lhsT = xt[c][:].rearrange("p (b n) -> p n b", n=NS)[:, n, :]
                        nc.tensor.matmul(pv[:], lhsT, twv[:, c * VH:(c + 1) * VH],
                                         start=(c == 0), stop=(c == NCHUNK - 1))
                        nc.tensor.matmul(pk[:], lhsT, twk[:, c * QL:(c + 1) * QL],
                                         start=(c == 0), stop=(c == NCHUNK - 1))
                    ov_sb = opool.tile([B, VH], f32)
                    nc.vector.tensor_copy(out=ov_sb[:], in_=pv[:])
                    ok_sb = opool.tile([B, QL], f32)
                    nc.vector.tensor_copy(out=ok_sb[:], in_=pk[:])
                    nc.scalar.dma_start(out=ovo[n], in_=ov_sb[:])
                    nc.scalar.dma_start(out=oko[n], in_=ok_sb[:])
                    if n + PRE < NS:
                        load_w(n + PRE)

    nc.compile()
    return nc


def _sigmoid(x):
    return 0.5 * (1.0 + np.tanh(0.5 * x))


def kernel(hidden_keys, hidden_values, query_bank, connectivity_scalars,
           ln_gamma, ln_beta, keys_w, keys_b, values_w, values_b,
           self_connection, _trace=False):
    global LAST_RESULTS
    hidden_keys = np.asarray(hidden_keys, np.float32)
    hidden_values = np.asarray(hidden_values, np.float32)
    query_bank = np.asarray(query_bank, np.float32)
    connectivity_scalars = np.asarray(connectivity_scalars, np.float32)
    ln_gamma = np.asarray(ln_gamma, np.float32)
    ln_beta = np.asarray(ln_beta, np.float32)
    keys_w = np.asarray(keys_w, np.float32)
    keys_b = np.asarray(keys_b, np.float32)
    values_w = np.asarray(values_w, np.float32)
    values_b = np.asarray(values_b, np.float32)
    sc_flag = bool(int(np.asarray(self_connection)))

    if sc_flag not in _PROGRAMS:
        _PROGRAMS[sc_flag] = _build(sc_flag)
    nc = _PROGRAMS[sc_flag]

    hkT16 = hidden_keys.transpose(2, 0, 1).astype(np.float16)      # (QL, B, S)
    hv16 = hidden_values.astype(np.float16)                        # (B, S, VH)
    qb = query_bank.reshape(H, N, QL)
    if np.all(ln_gamma == 1.0):
        kw, vw = keys_w, values_w
    else:
        kw = keys_w * ln_gamma[None, :, None]
        vw = values_w * ln_gamma[None, :, None]
    if sc_flag:
        sig = _sigmoid(connectivity_scalars[0])                    # (H, N, S)

    in_maps = []
    for i in range(NCORES):
        ns = i * NS
        m = {
            "hkT": hkT16,
            "hv": hv16,
            "qbT": (qb[:, ns:ns + NS, :].transpose(2, 0, 1).reshape(QL, H * NS)
                    / math.sqrt(QL)).astype(np.float16),
            "wv": vw[ns:ns + NS].reshape(NS, NCHUNK, 128, VH).transpose(0, 2, 1, 3)
                  .astype(np.float16).reshape(NS, 128, NCHUNK * VH),
            "wk": kw[ns:ns + NS].reshape(NS, NCHUNK, 128, QL).transpose(0, 2, 1, 3)
                  .astype(np.float16).reshape(NS, 128, NCHUNK * QL),
        }
        if sc_flag:
            m["sgT"] = sig[:, ns:ns + NS, :].transpose(2, 0, 1).reshape(
                S, H * NS).astype(np.float16)
        in_maps.append(m)

    res = bass_utils.run_bass_kernel_spmd(nc, in_maps, core_ids=list(range(NCORES)),
                                          trace=_trace)
    LAST_RESULTS = res

    ok_all = np.concatenate([np.asarray(res.results[i]["oko"]) for i in range(NCORES)], axis=0)
    ov_all = np.concatenate([np.asarray(res.results[i]["ovo"]) for i in range(NCORES)], axis=0)
    out_keys = np.ascontiguousarray(ok_all.transpose(1, 0, 2)).astype(np.float32)
    out_vals = np.ascontiguousarray(ov_all.transpose(1, 0, 2)).astype(np.float32)
    if np.any(ln_beta) or np.any(keys_b):
        out_keys = out_keys + (keys_b + np.einsum("i,nio->no", ln_beta, keys_w))[None]
    if np.any(ln_beta) or np.any(values_b):
        out_vals = out_vals + (values_b + np.einsum("i,nio->no", ln_beta, values_w))[None]
    return out_keys, out_vals


# revision 12
# speedup vs baseline: 1.1891x; 1.1891x over previous
import math
from contextlib import ExitStack

import numpy as np

from concourse import bass, tile, mybir, bacc, masks
from concourse import bass_utils

H, N, V, QL = 8, 512, 64, 64
VH = V * H
B, S = 32, 512
NCORES = 8
NS = N // NCORES            # 64 neurons per core
NCHUNK = VH // 128          # 4 contraction chunks
PRE = 12                    # weight prefetch depth (tiles)
EPS = 1e-12

_PROGRAMS = {}
LAST_RESULTS = None


def _build(self_conn: bool):
    f16, f32 = mybir.dt.float16, mybir.dt.float32
    Exp = mybir.ActivationFunctionType.Exp
    Sqrt = mybir.ActivationFunctionType.Sqrt

    nc = bacc.Bacc("TRN2", target_bir_lowering=False, debug=False, num_devices=NCORES)
    hkT = nc.dram_tensor("hkT", [QL, B, S], f16, kind="ExternalInput").ap()
    hv = nc.dram_tensor("hv", [B, S, H * 65], f16, kind="ExternalInput").ap()
    qbT = nc.dram_tensor("qbT", [QL, H * NS], f16, kind="ExternalInput").ap()
    sgT = None
    if self_conn:
        sgT = nc.dram_tensor("sgT", [S, H * NS], f16, kind="ExternalInput").ap()
    wv = nc.dram_tensor("wv", [NS, 128, NCHUNK * VH], f16, kind="ExternalInput").ap()
    wk = nc.dram_tensor("wk", [NS, 128, NCHUNK * QL], f16, kind="ExternalInput").ap()
    ovo = nc.dram_tensor("ovo", [NS, B, VH], f32, kind="ExternalOutput").ap()
    oko = nc.dram_tensor("oko", [NS, B, QL], f32, kind="ExternalOutput").ap()

    hvr = hv.rearrange("b (c p) v -> p (b c) v", p=128)

    with tile.TileContext(nc) as tc:
        with ExitStack() as ctx:
            singles = ctx.enter_context(tc.tile_pool(name="singles", bufs=1))
            hkp = ctx.enter_context(tc.tile_pool(name="hkp", bufs=2))
            apool = ctx.enter_context(tc.tile_pool(name="apool", bufs=2))
            epool = ctx.enter_context(tc.tile_pool(name="epool", bufs=2))
            txp = ctx.enter_context(tc.tile_pool(name="txp", bufs=2))
            xbp = ctx.enter_context(tc.tile_pool(name="xbp", bufs=2))
            smp = ctx.enter_context(tc.tile_pool(name="smp", bufs=2))
            wvp = ctx.enter_context(tc.tile_pool(name="wvp", bufs=PRE))
            wkp = ctx.enter_context(tc.tile_pool(name="wkp", bufs=PRE))
            opool = ctx.enter_context(tc.tile_pool(name="opool", bufs=4))

            qb_sb = singles.tile([QL, H * NS], f16)
            nc.sync.dma_start(out=qb_sb[:], in_=qbT[:])
            if self_conn:
                sg_sb = singles.tile([128, NCHUNK, H * NS], f16)
                nc.sync.dma_start(out=sg_sb[:], in_=sgT.rearrange("(c p) n -> p c n", p=128))
            ident = singles.tile([64, 64], f16)
            masks.make_identity(nc, ident[:])
            eps_sb = singles.tile([64, 1], f32)
            nc.vector.memset(eps_sb[:], EPS)
            xt = [singles.tile([128, B * NS], f16, name=f"xt{c}") for c in range(NCHUNK)]

            wtiles = {}

            def load_w(n):
                twv = wvp.tile([128, NCHUNK * VH], f16)
                nc.gpsimd.dma_start(out=twv[:], in_=wv[n])
                twk = wkp.tile([128, NCHUNK * QL], f16)
                nc.gpsimd.dma_start(out=twk[:], in_=wk[n])
                wtiles[n] = (twv, twk)

            for n in range(min(PRE, NS)):
                load_w(n)

            # ---- phase 1: attention + custom softmax + layernorm, per batch ----
            with tc.tile_pool(name="scps", bufs=2, space="PSUM") as scps, \
                 tc.tile_pool(name="atps", bufs=1, space="PSUM") as atps, \
                 tc.tile_pool(name="tpps", bufs=2, space="PSUM") as tpps:
                for b in range(B):
                    hk_sb = hkp.tile([QL, S], f16)
                    nc.sync.dma_start(out=hk_sb[:], in_=hkT[:, b, :])
                    a_sb = apool.tile([128, NCHUNK, 520], f16)
                    nc.sync.dma_start(out=a_sb[:], in_=hvr[:, b * 4:b * 4 + 4, :])

                    e_sb = epool.tile([128, NCHUNK, H * NS], f16)
                    for j in range(2):
                        sc = scps.tile([128, 2, H * NS], f32)
                        for jj in range(2):
                            c = 2 * j + jj
                            nc.tensor.matmul(sc[:, jj, :], hk_sb[:, c * 128:(c + 1) * 128],
                                             qb_sb[:], start=True, stop=True)
                        if self_conn:
                            tex = txp.tile([128, 2, H * NS], f16)
                            nc.scalar.activation(out=tex[:], in_=sc[:], func=Exp)
                            nc.vector.tensor_mul(e_sb[:, 2 * j:2 * j + 2, :], tex[:],
                                                 sg_sb[:, 2 * j:2 * j + 2, :])
                        else:
                            nc.scalar.activation(out=e_sb[:, 2 * j:2 * j + 2, :], in_=sc[:],
                                                 func=Exp)

                    at_a = atps.tile([64, 4, V + 1], f32)
                    at_b = atps.tile([64, 4, V + 1], f32)
                    for h in range(H):
                        at = at_a if h < 4 else at_b
                        for c in range(NCHUNK):
                            nc.tensor.matmul(at[:, h % 4, :],
                                             e_sb[:, c, h * NS:(h + 1) * NS],
                                             a_sb[:, c, 65 * h:65 * h + 65],
                                             start=(c == 0), stop=(c == NCHUNK - 1))

                    recs = smp.tile([64, H], f32)
                    nc.vector.reciprocal(recs[:, 0:4],
                                         at_a[:, :, V:V + 1].rearrange("p c o -> p (c o)"))
                    nc.vector.reciprocal(recs[:, 4:8],
                                         at_b[:, :, V:V + 1].rearrange("p c o -> p (c o)"))
                    x_b = xbp.tile([64, VH], f32)
                    xv = x_b[:].rearrange("p (v h) -> p h v", h=H)
                    r0 = recs[:]
                    ra = bass.AP(tensor=r0.tensor, offset=r0.offset, ap=[[H, 64], [1, 4], [0, V]])
                    rb = bass.AP(tensor=r0.tensor, offset=r0.offset + 4,
                                 ap=[[H, 64], [1, 4], [0, V]])
                    nc.vector.tensor_mul(xv[:, 0:4, :], at_a[:, :, 0:V], ra)
                    nc.vector.tensor_mul(xv[:, 4:8, :], at_b[:, :, 0:V], rb)

                    st6 = smp.tile([64, 6], f32)
                    mv = smp.tile([64, 2], f32)
                    nc.vector.bn_stats(st6[:], x_b[:])
                    nc.vector.bn_aggr(mv[:], st6[:])
                    rstd = smp.tile([64, 1], f32)
                    nc.scalar.activation(out=rstd[:], in_=mv[:, 1:2], func=Sqrt, bias=eps_sb[:])
                    nc.vector.reciprocal(rstd[:], rstd[:])
                    x16 = xbp.tile([64, VH], f16)
                    nc.vector.tensor_scalar(out=x16[:], in0=x_b[:], scalar1=mv[:, 0:1],
                                            scalar2=rstd[:], op0=mybir.AluOpType.subtract,
                                            op1=mybir.AluOpType.mult)
                    for c in range(NCHUNK):
                        pst = tpps.tile([128, 64], f16)
                        nc.tensor.transpose(pst[:], x16[:, c * 128:(c + 1) * 128], ident[:])
                        nc.vector.tensor_copy(out=xt[c][:, b * NS:(b + 1) * NS], in_=pst[:])

            # ---- phase 2: per-neuron linears ----
            with tc.tile_pool(name="pvps", bufs=2, space="PSUM") as pvps, \
                 tc.tile_pool(name="pkps", bufs=2, space="PSUM") as pkps:
                for n in range(NS):
                    twv, twk = wtiles.pop(n)
                    pv = pvps.tile([B, VH], f32)
                    pk = pkps.tile([B, QL], f32)
                    for c in range(NCHUNK):
                        lhsT = xt[c][:].rearrange("p (b n) -> p n b", n=NS)[:, n, :]
                        nc.tensor.matmul(pv[:], lhsT, twv[:, c * VH:(c + 1) * VH],
                                         start=(c == 0), stop=(c == NCHUNK - 1))
                        nc.tensor.matmul(pk[:], lhsT, twk[:, c * QL:(c + 1) * QL],
                                         start=(c == 0), stop=(c == NCHUNK - 1))
                    ov_sb = opool.tile([B, VH], f32)
                    nc.vector.tensor_copy(out=ov_sb[:], in_=pv[:])
                    ok_sb = opool.tile([B, QL], f32)
                    nc.vector.tensor_copy(out=ok_sb[:], in_=pk[:])
                    nc.scalar.dma_start(out=ovo[n], in_=ov_sb[:])
                    nc.scalar.dma_start(out=oko[n], in_=ok_sb[:])
                    if n + PRE < NS:
                        load_w(n + PRE)

    nc.compile()
    return nc


def _sigmoid(x):
    return 0.5 * (1.0 + np.tanh(0.5 * x))


def kernel(hidden_keys, hidden_values, query_bank, connectivity_scalars,
           ln_gamma, ln_beta, keys_w, keys_b, values_w, values_b,
           self_connection, _trace=False):
    global LAST_RESULTS
    hidden_keys = np.asarray(hidden_keys, np.float32)
    hidden_values = np.asarray(hidden_values, np.float32)
    query_bank = np.asarray(query_bank, np.float32)
    connectivity_scalars = np.asarray(connectivity_scalars, np.float32)
    ln_gamma = np.asarray(ln_gamma, np.float32)
    ln_beta = np.asarray(ln_beta, np.float32)
    keys_w = np.asarray(keys_w, np.float32)
    keys_b = np.asarray(keys_b, np.float32)
    values_w = np.asarray(values_w, np.float32)
    values_b = np.asarray(values_b, np.float32)
    sc_flag = bool(int(np.asarray(self_connection)))

    if sc_flag not in _PROGRAMS:
        _PROGRAMS[sc_flag] = _build(sc_flag)
    nc = _PROGRAMS[sc_flag]

    hkT16 = hidden_keys.transpose(2, 0, 1).astype(np.float16)      # (QL, B, S)
    hv16 = np.empty((B, S, H, V + 1), np.float16)                  # head-major + ones col
    hv16[..., :V] = hidden_values.reshape(B, S, V, H).transpose(0, 1, 3, 2)
    hv16[..., V] = 1.0
    hv16 = hv16.reshape(B, S, H * (V + 1))
    qb = query_bank.reshape(H, N, QL)
    if np.all(ln_gamma == 1.0):
        kw, vw = keys_w, values_w
    else:
        kw = keys_w * ln_gamma[None, :, None]
        vw = values_w * ln_gamma[None, :, None]
    if sc_flag:
        sig = _sigmoid(connectivity_scalars[0])                    # (H, N, S)

    in_maps = []
    for i in range(NCORES):
        ns = i * NS
        m = {
            "hkT": hkT16,
            "hv": hv16,
            "qbT": (qb[:, ns:ns + NS, :].transpose(2, 0, 1).reshape(QL, H * NS)
                    / math.sqrt(QL)).astype(np.float16),
            "wv": vw[ns:ns + NS].reshape(NS, NCHUNK, 128, VH).transpose(0, 2, 1, 3)
                  .astype(np.float16).reshape(NS, 128, NCHUNK * VH),
            "wk": kw[ns:ns + NS].reshape(NS, NCHUNK, 128, QL).transpose(0, 2, 1, 3)
                  .astype(np.float16).reshape(NS, 128, NCHUNK * QL),
        }
        if sc_flag:
            m["sgT"] = sig[:, ns:ns + NS, :].transpose(2, 0, 1).reshape(
                S, H * NS).astype(np.float16)
        in_maps.append(m)

    res = bass_utils.run_bass_kernel_spmd(nc, in_maps, core_ids=list(range(NCORES)),
                                          trace=_trace)
    LAST_RESULTS = res

    ok_all = np.concatenate([np.asarray(res.results[i]["oko"]) for i in range(NCORES)], axis=0)
    ov_all = np.concatenate([np.asarray(res.results[i]["ovo"]) for i in range(NCORES)], axis=0)
    out_keys = np.ascontiguousarray(ok_all.transpose(1, 0, 2)).astype(np.float32)
    out_vals = np.ascontiguousarray(ov_all.transpose(1, 0, 2)).astype(np.float32)
    if np.any(ln_beta) or np.any(keys_b):
        out_keys = out_keys + (keys_b + np.einsum("i,nio->no", ln_beta, keys_w))[None]
    if np.any(ln_beta) or np.any(values_b):
        out_vals = out_vals + (values_b + np.einsum("i,nio->no", ln_beta, values_w))[None]
    return out_keys, out_vals
